# revision 21
# baseline (speedup 1.0000x reference)
"""ALiBi positional bias (with contextual heads) on 8 TRN2 NeuronCores.

v3 architecture (row-split contextual + fp8 pattern masters):
  - Core c owns HALF of contextual head c//2 (rows 0:1024 for even c;
    rows 1024:2048 for odd c, fed row/col-REVERSED so the cumsum
    algebra maps onto the identical program — the left/right region
    formulas are flip-symmetric; host un-flips on gather).
  - Pattern heads (pure ALiBi -s*|i-j|) are flip-symmetric, so every
    core also streams one FULL pattern head (outB, head 4+c) and one
    HALF pattern head (outC, head 12+c//2) from host-precomputed fp8
    (e4m3, 240-max; heads 4/5 master pre-scaled by 1/2, host scales
    back) masters resident in SBUF — pure DMA, batched 4 tiles per
    instruction on the scalar (ACT) HWDGE queue.
  - Contextual pipeline per tile t (rt=128t): PE matmul (bf16 qT/kT,
    host pre-transposed) -> ACT sigmoid (bf16 sig) -> scan to
    gz[:,1:2049] f32 (gz[:,0]=0) on DVE (6 tiles) or GpSimd (2 tiles)
    -> GpSimd diag extracts dg_s=s*gz[p,rt+p+1], dgm1_ns=-s*gz[p,rt+p]
    and d1 (right-formula diag block) -> ACT region ops (bf16 out)
    -> DVE copy_predicated diag merge -> sync-queue DMA out.
  - Output dtypes: contextual bf16, pattern fp8-as-uint8 (device only
    moves bytes; ml_dtypes does all fp8 encode/decode host-side).
    Host applies the prefix-cross overwrite (two thin strips) and
    upcasts to f32. Total simulated rel err ~1.3e-2 vs 2e-2 budget.
"""

import os
import sys
import math
import numpy as np

if "/opt/trn_rl_repo" not in sys.path:
    sys.path.insert(0, "/opt/trn_rl_repo")

from contextlib import ExitStack

import ml_dtypes

import concourse.bass as bass
import concourse.tile as tile
from concourse import bacc, mybir, masks
from concourse.ap import AP
from concourse.bass_utils import run_bass_kernel_spmd

F32 = mybir.dt.float32
BF16 = mybir.dt.bfloat16
U8 = mybir.dt.uint8
ALU = mybir.AluOpType
ACTF = mybir.ActivationFunctionType

S = 2048        # i = j = sequence length
HS = S // 2     # rows per half head
D = 64          # head dim
NCORES = 8
P = 128         # SBUF partitions
HT = HS // P    # 8 row tiles per half head
PREFIX = 16
TOTAL_HEADS = 16

GP_SCAN_TILES = ()   # walrus rejects the scan opcode on Pool: DVE only
FP8_SCALE = {4: 0.5, 5: 0.5}  # heads whose master is pre-scaled (e4m3 max 240)

PROFILE = False       # test.py sets True to capture an NTFF profile
LAST_RESULT = None    # BassKernelResults of the most recent run

_NC_CACHE = None


DEBUG_LEVEL = int(os.environ.get("K_DEBUG_LEVEL", "9"))


def _build_nc():
    nc = bacc.Bacc(
        "TRN2",
        target_bir_lowering=False,
        debug=False,
        enable_asserts=False,
        num_devices=NCORES,
    )
    qT_ext = nc.dram_tensor("qT", [D, HS], BF16, kind="ExternalInput").ap()
    kT_ext = nc.dram_tensor("kT", [D, S], BF16, kind="ExternalInput").ap()
    sl_ext = nc.dram_tensor("sl", [1, 2], F32, kind="ExternalInput").ap()
    mB_ext = nc.dram_tensor("mB", [P, 2 * S], U8, kind="ExternalInput").ap()
    mC_ext = nc.dram_tensor("mC", [P, 2 * S], U8, kind="ExternalInput").ap()
    outA_ext = nc.dram_tensor("outA", [HS, S], BF16, kind="ExternalOutput").ap()
    outB_ext = nc.dram_tensor("outB", [S, S], U8, kind="ExternalOutput").ap()
    outC_ext = nc.dram_tensor("outC", [HS, S], U8, kind="ExternalOutput").ap()

    with tile.TileContext(nc) as tc, ExitStack() as ctx:
        const = ctx.enter_context(tc.tile_pool(name="const", bufs=1))

        # ---- inputs: kT/qT first — they gate the first matmul/sigmoid ----
        kT = const.tile([D, S], BF16, tag="kT", name="kT")
        qT = const.tile([D, HS], BF16, tag="qT", name="qT")
        nc.sync.dma_start(kT[:], kT_ext[:])
        nc.sync.dma_start(qT[:], qT_ext[:])
        sl_raw = const.tile([1, 2], F32, tag="sl_raw", name="sl_raw")
        nc.sync.dma_start(sl_raw[:], sl_ext[:])
        mB = const.tile([P, 2 * S], U8, tag="mB", name="mB")
        mC = const.tile([P, 2 * S], U8, tag="mC", name="mC")
        nc.sync.dma_start(mB[:], mB_ext[:])
        nc.sync.dma_start(mC[:], mC_ext[:])

        slv = const.tile([P, 2], F32, tag="slv", name="slv")
        nc.gpsimd.partition_broadcast(slv[:], sl_raw[:])
        svA = slv[:, 0:1]   # +s
        snA = slv[:, 1:2]   # -s

        # keep the prologue OFF DVE: scale identities on ACT, cast on ACT
        ident = const.tile([P, P], F32, tag="ident", name="ident")
        masks.make_identity(nc, ident[:])
        i_s = const.tile([P, P], F32, tag="i_s", name="i_s")
        nc.scalar.activation(i_s[:], ident[:], ACTF.Identity, scale=svA)
        i_ns = const.tile([P, P], F32, tag="i_ns", name="i_ns")
        nc.scalar.activation(i_ns[:], ident[:], ACTF.Identity, scale=snA)

        u128f = const.tile([P, P], F32, tag="u128f", name="u128f")
        masks.make_upper_triangular(nc, u128f[:], val=1.0, diag=False)
        u128 = const.tile([P, P], U8, tag="u128", name="u128")
        nc.scalar.copy(u128[:], u128f[:])

        zeros = const.tile([P, S], BF16, tag="zeros", name="zeros")
        nc.gpsimd.memset(zeros[:], 0.0)

        # ---- pattern streams: batched multi-tile DMAs on the scalar queue ----
        # master view for row-tile T is m[:, S-128T : 2S-128T]; group GB
        # consecutive tiles into one DMA via a [p, T, c] access pattern
        # (SBUF side walks windows with stride -128).
        def emit_master_stream(m_tile, out_ext_ap, nt, group):
            v = m_tile[:]
            for t0 in range(0, nt, group):
                g = min(group, nt - t0)
                in_ap = AP(
                    tensor=v.tensor,
                    offset=v.offset + (S - P * t0),
                    ap=[[2 * S, P], [-P, g], [1, S]],
                )
                out_ap = AP(
                    tensor=out_ext_ap.tensor,
                    offset=out_ext_ap.offset + t0 * P * S,
                    ap=[[S, P], [P * S, g], [1, S]],
                )
                nc.sync.dma_start(out_ap, in_ap)

        emit_master_stream(mB, outB_ext, 2 * HT, 4)
        emit_master_stream(mC, outC_ext, HT, 4)

        with (
            tc.tile_pool(name="psum_s", bufs=2, space=bass.MemorySpace.PSUM) as psS,
            tc.tile_pool(name="sigp", bufs=3) as sigp,
            tc.tile_pool(name="gp", bufs=3) as gzp,
            tc.tile_pool(name="outp", bufs=3) as outp,
            tc.tile_pool(name="smallp", bufs=3) as sp,
        ):
            state = {}

            def emit_mm_sig(t):
                rt = t * P
                ps = psS.tile([P, S], F32, tag="s", name="s")
                for n4 in range(4):
                    c0 = n4 * 512
                    nc.tensor.matmul(
                        ps[:, c0 : c0 + 512],
                        qT[:, rt : rt + P],
                        kT[:, c0 : c0 + 512],
                        start=True,
                        stop=True,
                    )
                sig = sigp.tile([P, S], BF16, tag="sig", name="sig")
                nc.scalar.activation(sig[:], ps[:], ACTF.Sigmoid, scale=0.125)
                state[t] = [sig]

            def emit_scan(t):
                (sig,) = state[t]
                gz = gzp.tile([P, S + 1], F32, tag="gz", name="gz")
                nc.gpsimd.memset(gz[:, 0:1], 0.0)
                nc.vector.tensor_tensor_scan(
                    gz[:, 1 : S + 1], sig[:], zeros[:], 0.0, ALU.add, ALU.bypass
                )
                state[t] = [gz]

            def emit_diag(t):
                # dgm1_ns[p] = -s*gz[p, rt+p];  dg_s[p] = s*gz[p, rt+p+1]
                # GpSimd mults overlap the NEXT tile's scan; DVE reduces
                # follow that scan in DVE program order (latency hidden).
                rt = t * P
                (gz,) = state[t]
                scr = sp.tile([P, P], F32, tag="scr", name="scr")
                scr2 = sp.tile([P, P], F32, tag="scr2", name="scr2")
                dgm1_ns = sp.tile([P, 1], F32, tag="dgm1", name="dgm1")
                dg_s = sp.tile([P, 1], F32, tag="dgs", name="dgs")
                nc.gpsimd.tensor_tensor(
                    scr[:], gz[:, rt : rt + P], i_ns[:], op=ALU.mult
                )
                nc.gpsimd.tensor_tensor(
                    scr2[:], gz[:, rt + 1 : rt + P + 1], i_s[:], op=ALU.mult
                )
                nc.vector.tensor_reduce(
                    dgm1_ns[:], scr[:], mybir.AxisListType.X, ALU.add
                )
                nc.vector.tensor_reduce(
                    dg_s[:], scr2[:], mybir.AxisListType.X, ALU.add
                )
                state[t] = [gz, dgm1_ns, dg_s]

            def emit_tail(t):
                rt = t * P
                gz, dgm1_ns, dg_s = state.pop(t)
                out_t = outp.tile([P, S], BF16, tag="out", name="out")
                # diag-block right-formula values (merged via copy_pred)
                d1 = sp.tile([P, P], BF16, tag="d1", name="d1")
                nc.scalar.activation(
                    d1[:], gz[:, rt + 1 : rt + P + 1],
                    ACTF.Identity, bias=dg_s[:], scale=snA,
                )
                # right of the diag block: -s*H[c] + s*H[r]
                nc.scalar.activation(
                    out_t[:, rt + P : S], gz[:, rt + P + 1 : S + 1],
                    ACTF.Identity, bias=dg_s[:], scale=snA,
                )
                # left incl diag cols 0..rt+127: s*H[c-1] - s*H[r-1]
                nc.scalar.activation(
                    out_t[:, 0 : rt + P], gz[:, 0 : rt + P],
                    ACTF.Identity, bias=dgm1_ns[:], scale=svA,
                )
                # diag-block upper part gets right-formula values
                nc.vector.copy_predicated(out_t[:, rt : rt + P], u128[:], d1[:])
                nc.sync.dma_start(outA_ext[rt : rt + P, :], out_t[:])

            emit_mm_sig(0)
            emit_scan(0)
            for t in range(1, HT):
                emit_mm_sig(t)
                emit_scan(t)
                emit_diag(t - 1)
                emit_tail(t - 1)
            emit_diag(HT - 1)
            emit_tail(HT - 1)

    nc.compile()
    return nc


def _get_nc():
    global _NC_CACHE
    if _NC_CACHE is None:
        _NC_CACHE = _build_nc()
    return _NC_CACHE


def _alibi_slopes(heads: int) -> np.ndarray:
    def pow2_slopes(n):
        start = 2 ** (-(2 ** (-(math.log2(n) - 3))))
        return [start * start**i for i in range(n)]

    if math.log2(heads).is_integer():
        return np.array(pow2_slopes(heads), dtype=np.float32)
    closest = 2 ** math.floor(math.log2(heads))
    return np.array(
        pow2_slopes(closest) + pow2_slopes(2 * closest)[0::2][: heads - closest],
        dtype=np.float32,
    )


def _master(slope_scaled: float) -> np.ndarray:
    # m[p, x] = -s*|x - S - p|, fp8 e4m3 bytes
    x = np.arange(2 * S, dtype=np.float32)[None, :]
    p = np.arange(P, dtype=np.float32)[:, None]
    m = (-slope_scaled * np.abs(x - S - p)).astype(np.float32)
    return m.astype(ml_dtypes.float8_e4m3).view(np.uint8)


def kernel(q, k, cross_attn_bias, i, j, offset, prefix) -> np.ndarray:
    global LAST_RESULT
    q = np.asarray(q, dtype=np.float32)
    k = np.asarray(k, dtype=np.float32)
    cab = np.asarray(cross_attn_bias, dtype=np.float32).reshape(TOTAL_HEADS)
    assert int(i) == S and int(j) == S and int(offset) == 0 and int(prefix) == PREFIX
    assert q.shape == (1, TOTAL_HEADS, S, D) and k.shape == (1, TOTAL_HEADS, S, D)

    slopes = _alibi_slopes(TOTAL_HEADS)
    bf = ml_dtypes.bfloat16

    in_maps = []
    for c in range(NCORES):
        hA, high = c // 2, c % 2
        hB = 4 + c
        hC = 12 + c // 2
        if high:
            qrows = q[0, hA, HS:S][::-1]
            krows = k[0, hA][::-1]
        else:
            qrows = q[0, hA, 0:HS]
            krows = k[0, hA]
        sB = float(slopes[hB]) * FP8_SCALE.get(hB, 1.0)
        sC = float(slopes[hC]) * FP8_SCALE.get(hC, 1.0)
        in_maps.append(
            {
                "qT": np.ascontiguousarray(qrows.T.astype(bf)),
                "kT": np.ascontiguousarray(krows.T.astype(bf)),
                "sl": np.array([[slopes[hA], -slopes[hA]]], np.float32),
                "mB": _master(sB),
                "mC": _master(sC),
            }
        )

    res = run_bass_kernel_spmd(
        _get_nc(), in_maps, list(range(NCORES)), trace=PROFILE
    )
    LAST_RESULT = res

    full = np.empty((1, TOTAL_HEADS, S, S), dtype=np.float32)
    f8 = ml_dtypes.float8_e4m3
    for c in range(NCORES):
        hA, high = c // 2, c % 2
        hB = 4 + c
        hC = 12 + c // 2
        r = res.results[c]
        oa = np.asarray(r["outA"]).astype(np.float32)
        ob = np.asarray(r["outB"]).view(f8).astype(np.float32)
        oc = np.asarray(r["outC"]).view(f8).astype(np.float32)
        if FP8_SCALE.get(hB):
            ob *= 1.0 / FP8_SCALE[hB]
        if FP8_SCALE.get(hC):
            oc *= 1.0 / FP8_SCALE[hC]
        if high:
            full[0, hA, HS:S] = oa[::-1, ::-1]
            full[0, hC, HS:S] = oc[::-1, ::-1]
        else:
            full[0, hA, 0:HS] = oa
            full[0, hC, 0:HS] = oc
        full[0, hB] = ob

    # prefix cross-attention overwrite (two thin strips per head)
    cabv = cab.reshape(TOTAL_HEADS, 1, 1)
    full[0, :, 0:PREFIX, PREFIX:] = cabv
    full[0, :, PREFIX:, 0:PREFIX] = cabv
    return full


# revision 25
# speedup vs baseline: 1.0212x; 1.0212x over previous
"""ALiBi positional bias (with contextual heads) on 8 TRN2 NeuronCores.

v3 architecture (row-split contextual + fp8 pattern masters):
  - Core c owns HALF of contextual head c//2 (rows 0:1024 for even c;
    rows 1024:2048 for odd c, fed row/col-REVERSED so the cumsum
    algebra maps onto the identical program — the left/right region
    formulas are flip-symmetric; host un-flips on gather).
  - Pattern heads (pure ALiBi -s*|i-j|) are flip-symmetric, so every
    core also streams one FULL pattern head (outB, head 4+c) and one
    HALF pattern head (outC, head 12+c//2) from host-precomputed fp8
    (e4m3, 240-max; heads 4/5 master pre-scaled by 1/2, host scales
    back) masters resident in SBUF — pure DMA, batched 4 tiles per
    instruction on the scalar (ACT) HWDGE queue.
  - Contextual pipeline per tile t (rt=128t): PE matmul (bf16 qT/kT,
    host pre-transposed) -> ACT sigmoid (bf16 sig) -> scan to
    gz[:,1:2049] f32 (gz[:,0]=0) on DVE (6 tiles) or GpSimd (2 tiles)
    -> GpSimd diag extracts dg_s=s*gz[p,rt+p+1], dgm1_ns=-s*gz[p,rt+p]
    and d1 (right-formula diag block) -> ACT region ops (bf16 out)
    -> DVE copy_predicated diag merge -> sync-queue DMA out.
  - Output dtypes: contextual bf16, pattern fp8-as-uint8 (device only
    moves bytes; ml_dtypes does all fp8 encode/decode host-side).
    Host applies the prefix-cross overwrite (two thin strips) and
    upcasts to f32. Total simulated rel err ~1.3e-2 vs 2e-2 budget.
"""

import os
import sys
import math
import numpy as np

if "/opt/trn_rl_repo" not in sys.path:
    sys.path.insert(0, "/opt/trn_rl_repo")

from contextlib import ExitStack

import ml_dtypes

import concourse.bass as bass
import concourse.tile as tile
from concourse import bacc, mybir, masks
from concourse.ap import AP
from concourse.bass_utils import run_bass_kernel_spmd

F32 = mybir.dt.float32
BF16 = mybir.dt.bfloat16
U8 = mybir.dt.uint8
ALU = mybir.AluOpType
ACTF = mybir.ActivationFunctionType

S = 2048        # i = j = sequence length
HS = S // 2     # rows per half head
D = 64          # head dim
NCORES = 8
P = 128         # SBUF partitions
HT = HS // P    # 8 row tiles per half head
PREFIX = 16
TOTAL_HEADS = 16

GP_SCAN_TILES = ()   # walrus rejects the scan opcode on Pool: DVE only
FP8_SCALE = {4: 0.5, 5: 0.5}  # heads whose master is pre-scaled (e4m3 max 240)

PROFILE = False       # test.py sets True to capture an NTFF profile
LAST_RESULT = None    # BassKernelResults of the most recent run

_NC_CACHE = None


DEBUG_LEVEL = int(os.environ.get("K_DEBUG_LEVEL", "9"))


def _build_nc():
    nc = bacc.Bacc(
        "TRN2",
        target_bir_lowering=False,
        debug=False,
        enable_asserts=False,
        num_devices=NCORES,
    )
    qT_ext = nc.dram_tensor("qT", [D, HS], BF16, kind="ExternalInput").ap()
    kT_ext = nc.dram_tensor("kT", [D, S], BF16, kind="ExternalInput").ap()
    sl_ext = nc.dram_tensor("sl", [1, 2], F32, kind="ExternalInput").ap()
    mB_ext = nc.dram_tensor("mB", [P, 2 * S], U8, kind="ExternalInput").ap()
    mC_ext = nc.dram_tensor("mC", [P, 2 * S], U8, kind="ExternalInput").ap()
    outA_ext = nc.dram_tensor("outA", [HS, S], BF16, kind="ExternalOutput").ap()
    outB_ext = nc.dram_tensor("outB", [S, S], U8, kind="ExternalOutput").ap()
    outC_ext = nc.dram_tensor("outC", [HS, S], U8, kind="ExternalOutput").ap()

    with tile.TileContext(nc) as tc, ExitStack() as ctx:
        const = ctx.enter_context(tc.tile_pool(name="const", bufs=1))

        # ---- inputs: kT/qT first — they gate the first matmul/sigmoid ----
        kT = const.tile([D, S], BF16, tag="kT", name="kT")
        qT = const.tile([D, HS], BF16, tag="qT", name="qT")
        nc.sync.dma_start(kT[:], kT_ext[:])
        nc.sync.dma_start(qT[:], qT_ext[:])
        sl_raw = const.tile([1, 2], F32, tag="sl_raw", name="sl_raw")
        nc.sync.dma_start(sl_raw[:], sl_ext[:])
        mB = const.tile([P, 2 * S], U8, tag="mB", name="mB")
        mC = const.tile([P, 2 * S], U8, tag="mC", name="mC")
        nc.sync.dma_start(mB[:], mB_ext[:])
        nc.sync.dma_start(mC[:], mC_ext[:])

        # constants are built LATE (emit_consts below) so tile 0's
        # matmul/sigmoid/scan lead each engine's in-order queue; these
        # dicts are filled by emit_consts before emit_diag(0) needs them.
        C = {}

        def emit_consts():
            slv = const.tile([P, 2], F32, tag="slv", name="slv")
            nc.gpsimd.partition_broadcast(slv[:], sl_raw[:])
            C["svA"] = slv[:, 0:1]   # +s
            C["snA"] = slv[:, 1:2]   # -s
            ident = const.tile([P, P], F32, tag="ident", name="ident")
            masks.make_identity(nc, ident[:])
            i_s = const.tile([P, P], F32, tag="i_s", name="i_s")
            nc.scalar.activation(i_s[:], ident[:], ACTF.Identity, scale=C["svA"])
            i_ns = const.tile([P, P], F32, tag="i_ns", name="i_ns")
            nc.scalar.activation(i_ns[:], ident[:], ACTF.Identity, scale=C["snA"])
            u128f = const.tile([P, P], F32, tag="u128f", name="u128f")
            masks.make_upper_triangular(nc, u128f[:], val=1.0, diag=False)
            u128 = const.tile([P, P], U8, tag="u128", name="u128")
            nc.scalar.copy(u128[:], u128f[:])
            C["i_s"], C["i_ns"], C["u128"] = i_s, i_ns, u128

        # ---- pattern streams: batched multi-tile DMAs on the scalar queue ----
        # master view for row-tile T is m[:, S-128T : 2S-128T]; group GB
        # consecutive tiles into one DMA via a [p, T, c] access pattern
        # (SBUF side walks windows with stride -128).
        def emit_master_stream(m_tile, out_ext_ap, nt, group):
            v = m_tile[:]
            for t0 in range(0, nt, group):
                g = min(group, nt - t0)
                in_ap = AP(
                    tensor=v.tensor,
                    offset=v.offset + (S - P * t0),
                    ap=[[2 * S, P], [-P, g], [1, S]],
                )
                out_ap = AP(
                    tensor=out_ext_ap.tensor,
                    offset=out_ext_ap.offset + t0 * P * S,
                    ap=[[S, P], [P * S, g], [1, S]],
                )
                nc.sync.dma_start(out_ap, in_ap)

        emit_master_stream(mB, outB_ext, 2 * HT, 4)
        emit_master_stream(mC, outC_ext, HT, 4)

        with (
            tc.tile_pool(name="psum_s", bufs=2, space=bass.MemorySpace.PSUM) as psS,
            tc.tile_pool(name="sigp", bufs=3) as sigp,
            tc.tile_pool(name="gp", bufs=3) as gzp,
            tc.tile_pool(name="outp", bufs=3) as outp,
            tc.tile_pool(name="smallp", bufs=3) as sp,
        ):
            state = {}

            def emit_mm_sig(t):
                rt = t * P
                ps = psS.tile([P, S], F32, tag="s", name="s")
                for n4 in range(4):
                    c0 = n4 * 512
                    nc.tensor.matmul(
                        ps[:, c0 : c0 + 512],
                        qT[:, rt : rt + P],
                        kT[:, c0 : c0 + 512],
                        start=True,
                        stop=True,
                    )
                sig = sigp.tile([P, S], BF16, tag="sig", name="sig")
                nc.scalar.activation(sig[:], ps[:], ACTF.Sigmoid, scale=0.125)
                state[t] = [sig]

            def emit_scan(t):
                (sig,) = state[t]
                gz = gzp.tile([P, S + 1], F32, tag="gz", name="gz")
                nc.gpsimd.memset(gz[:, 0:1], 0.0)
                # op1=bypass ignores data1 — pass sig again (no extra dep)
                nc.vector.tensor_tensor_scan(
                    gz[:, 1 : S + 1], sig[:], sig[:], 0.0, ALU.add, ALU.bypass
                )
                state[t] = [gz]

            def emit_diag(t):
                # dgm1_ns[p] = -s*gz[p, rt+p];  dg_s[p] = s*gz[p, rt+p+1]
                # GpSimd mults overlap the NEXT tile's scan; the single
                # fused DVE reduce follows that scan in DVE program order.
                rt = t * P
                (gz,) = state[t]
                scr = sp.tile([P, 2 * P], F32, tag="scr", name="scr")
                dg2 = sp.tile([P, 2], F32, tag="dg2", name="dg2")
                nc.gpsimd.tensor_tensor(
                    scr[:, 0:P], gz[:, rt : rt + P], C["i_ns"][:], op=ALU.mult
                )
                nc.gpsimd.tensor_tensor(
                    scr[:, P : 2 * P], gz[:, rt + 1 : rt + P + 1], C["i_s"][:],
                    op=ALU.mult,
                )
                nc.vector.tensor_reduce(
                    dg2[:], scr[:].rearrange("p (a b) -> p a b", a=2),
                    mybir.AxisListType.X, ALU.add,
                )
                state[t] = [gz, dg2[:, 0:1], dg2[:, 1:2]]

            def emit_tail(t):
                rt = t * P
                gz, dgm1_ns, dg_s = state.pop(t)
                out_t = outp.tile([P, S], BF16, tag="out", name="out")
                # diag-block right-formula values (merged via copy_pred)
                d1 = sp.tile([P, P], BF16, tag="d1", name="d1")
                nc.scalar.activation(
                    d1[:], gz[:, rt + 1 : rt + P + 1],
                    ACTF.Identity, bias=dg_s, scale=C["snA"],
                )
                # right of the diag block: -s*H[c] + s*H[r]
                nc.scalar.activation(
                    out_t[:, rt + P : S], gz[:, rt + P + 1 : S + 1],
                    ACTF.Identity, bias=dg_s, scale=C["snA"],
                )
                # left incl diag cols 0..rt+127: s*H[c-1] - s*H[r-1]
                nc.scalar.activation(
                    out_t[:, 0 : rt + P], gz[:, 0 : rt + P],
                    ACTF.Identity, bias=dgm1_ns, scale=C["svA"],
                )
                # diag-block upper part gets right-formula values
                nc.vector.copy_predicated(
                    out_t[:, rt : rt + P], C["u128"][:], d1[:]
                )
                nc.sync.dma_start(outA_ext[rt : rt + P, :], out_t[:])

            emit_mm_sig(0)
            emit_scan(0)
            emit_consts()   # constant factory queues BEHIND tile 0's ops
            for t in range(1, HT):
                emit_mm_sig(t)
                emit_scan(t)
                emit_diag(t - 1)
                emit_tail(t - 1)
            emit_diag(HT - 1)
            emit_tail(HT - 1)

    nc.compile()
    return nc


def _get_nc():
    global _NC_CACHE
    if _NC_CACHE is None:
        _NC_CACHE = _build_nc()
    return _NC_CACHE


def _alibi_slopes(heads: int) -> np.ndarray:
    def pow2_slopes(n):
        start = 2 ** (-(2 ** (-(math.log2(n) - 3))))
        return [start * start**i for i in range(n)]

    if math.log2(heads).is_integer():
        return np.array(pow2_slopes(heads), dtype=np.float32)
    closest = 2 ** math.floor(math.log2(heads))
    return np.array(
        pow2_slopes(closest) + pow2_slopes(2 * closest)[0::2][: heads - closest],
        dtype=np.float32,
    )


def _master(slope_scaled: float) -> np.ndarray:
    # m[p, x] = -s*|x - S - p|, fp8 e4m3 bytes
    x = np.arange(2 * S, dtype=np.float32)[None, :]
    p = np.arange(P, dtype=np.float32)[:, None]
    m = (-slope_scaled * np.abs(x - S - p)).astype(np.float32)
    return m.astype(ml_dtypes.float8_e4m3).view(np.uint8)


def kernel(q, k, cross_attn_bias, i, j, offset, prefix) -> np.ndarray:
    global LAST_RESULT
    q = np.asarray(q, dtype=np.float32)
    k = np.asarray(k, dtype=np.float32)
    cab = np.asarray(cross_attn_bias, dtype=np.float32).reshape(TOTAL_HEADS)
    assert int(i) == S and int(j) == S and int(offset) == 0 and int(prefix) == PREFIX
    assert q.shape == (1, TOTAL_HEADS, S, D) and k.shape == (1, TOTAL_HEADS, S, D)

    slopes = _alibi_slopes(TOTAL_HEADS)
    bf = ml_dtypes.bfloat16

    in_maps = []
    for c in range(NCORES):
        hA, high = c // 2, c % 2
        hB = 4 + c
        hC = 12 + c // 2
        if high:
            qrows = q[0, hA, HS:S][::-1]
            krows = k[0, hA][::-1]
        else:
            qrows = q[0, hA, 0:HS]
            krows = k[0, hA]
        sB = float(slopes[hB]) * FP8_SCALE.get(hB, 1.0)
        sC = float(slopes[hC]) * FP8_SCALE.get(hC, 1.0)
        in_maps.append(
            {
                "qT": np.ascontiguousarray(qrows.T.astype(bf)),
                "kT": np.ascontiguousarray(krows.T.astype(bf)),
                "sl": np.array([[slopes[hA], -slopes[hA]]], np.float32),
                "mB": _master(sB),
                "mC": _master(sC),
            }
        )

    res = run_bass_kernel_spmd(
        _get_nc(), in_maps, list(range(NCORES)), trace=PROFILE
    )
    LAST_RESULT = res

    full = np.empty((1, TOTAL_HEADS, S, S), dtype=np.float32)
    f8 = ml_dtypes.float8_e4m3
    for c in range(NCORES):
        hA, high = c // 2, c % 2
        hB = 4 + c
        hC = 12 + c // 2
        r = res.results[c]
        oa = np.asarray(r["outA"]).astype(np.float32)
        ob = np.asarray(r["outB"]).view(f8).astype(np.float32)
        oc = np.asarray(r["outC"]).view(f8).astype(np.float32)
        if FP8_SCALE.get(hB):
            ob *= 1.0 / FP8_SCALE[hB]
        if FP8_SCALE.get(hC):
            oc *= 1.0 / FP8_SCALE[hC]
        if high:
            full[0, hA, HS:S] = oa[::-1, ::-1]
            full[0, hC, HS:S] = oc[::-1, ::-1]
        else:
            full[0, hA, 0:HS] = oa
            full[0, hC, 0:HS] = oc
        full[0, hB] = ob

    # prefix cross-attention overwrite (two thin strips per head)
    cabv = cab.reshape(TOTAL_HEADS, 1, 1)
    full[0, :, 0:PREFIX, PREFIX:] = cabv
    full[0, :, PREFIX:, 0:PREFIX] = cabv
    return full
